# revision 31
# baseline (speedup 1.0000x reference)
"""Trainium2 Bass kernel for nn_AttDual: dual-attention MIL pooling head.

Computation (see reference):
  K = chain(features, key_*)    ; chain = Linear -> LayerNorm -> GELU(erf)
  V = chain(K, value_*)
  Q = chain(K, query_*)
  top_idx = argmax(c, axis=0)   ; q_max = chain(K[top_idx], query_*)  (== Q[top_idx])
  A = softmax(Q @ q_max.T / 32, axis=0)   (column softmax over all N)
  B = A.T @ V ;  C = einsum('ik,oik->o', B, head_w) + head_b
Returns (C [1,7], A [N,7], B [1,7,1024]).

Distribution: data-parallel over N across 8 NeuronCores. Collectives:
one AllGather of [local-argmax value | selected K row] per class (each core
then picks the winners locally), and one AllReduce of [colsum | B].

Structure: pass A streams features and computes K tiles (K^T spilled to
DRAM) plus the local-argmax selection; pass B streams K^T back and fuses
the V/Q chains with logits -> exp -> B-accumulation per tile, lagged a few
tiles so the q_max chain hides under the head of pass B.

Matmuls in bf16 with f32 PSUM accumulation. LayerNorm stats via bn_stats;
rsqrt via DVE bit-trick + 2 Newton steps (keeps ScalarE on the Gelu table
through both passes; one switch to the exp table for the softmax).
"""
import numpy as np

import concourse.bacc as bacc
import concourse.tile as tile
import concourse.mybir as mybir
from concourse.bass_utils import run_bass_kernel_spmd

F32 = mybir.dt.float32
I32 = mybir.dt.int32
BF16 = mybir.dt.bfloat16
AF = mybir.ActivationFunctionType
ALU = mybir.AluOpType

N_CORES = 8
D = 1024
DK = 8          # d / 128
C7 = 7
EPS = 1e-5
NEG_BIG = -1.0e30

_BUILD_CACHE: dict = {}


def build_graph(rs: int, n_cores: int = N_CORES):
    """rs = rows per core (real). Tiles of 128 rows; last tile partial."""
    rt = (rs + 127) // 128
    vr_last = rs - (rt - 1) * 128
    ncp = n_cores * C7

    nc = bacc.Bacc("TRN2", target_bir_lowering=False, debug=False,
                   num_devices=n_cores)

    xt_ext = nc.declare_dram_parameter("xt", [rt, 128, DK, 128], BF16, isOutput=False)
    c_ext = nc.declare_dram_parameter("call", [128, rt, C7], F32, isOutput=False)
    wk_ext = nc.declare_dram_parameter("wk", [DK, 128, D], BF16, isOutput=False)
    wv_ext = nc.declare_dram_parameter("wv", [DK, 128, D], BF16, isOutput=False)
    wq_ext = nc.declare_dram_parameter("wq", [DK, 128, D], BF16, isOutput=False)
    kb_ext = nc.declare_dram_parameter("kb", [1, D], BF16, isOutput=False)
    vb_ext = nc.declare_dram_parameter("vb", [1, D], BF16, isOutput=False)
    qb_ext = nc.declare_dram_parameter("qb", [1, D], BF16, isOutput=False)
    hwt_ext = nc.declare_dram_parameter("hwt", [128, DK, C7, C7], F32, isOutput=False)
    hb_ext = nc.declare_dram_parameter("hb", [1, C7], F32, isOutput=False)
    eye_ext = nc.declare_dram_parameter("eye", [ncp, C7], F32, isOutput=False)

    a_ext = nc.declare_dram_parameter("A_out", [128, rt, C7], F32, isOutput=True)
    b_ext = nc.declare_dram_parameter("B_out", [C7, D], F32, isOutput=True)
    co_ext = nc.declare_dram_parameter("C_out", [1, C7], F32, isOutput=True)

    rg = [list(range(n_cores))]

    with tile.TileContext(nc) as tc:
        with (
            tc.tile_pool(name="const", bufs=1) as cp,
            tc.tile_pool(name="io", bufs=4) as iop,
            tc.tile_pool(name="act", bufs=2) as ap_,
            tc.tile_pool(name="qtp", bufs=20) as qtp,
            tc.tile_pool(name="vp", bufs=20) as vp,
            tc.tile_pool(name="sm", bufs=3) as smp,
            tc.tile_pool(name="ps_h", bufs=4, space="PSUM") as ps_h,
            tc.tile_pool(name="ps_acc", bufs=1, space="PSUM") as ps_acc,
            tc.tile_pool(name="ps_sm", bufs=2, space="PSUM") as ps_sm,
            tc.tile_pool(name="dram", bufs=1, space="DRAM") as dram,
        ):
            # ---------------- constants / pinned ----------------
            c_all = cp.tile([128, rt, C7], F32)
            nc.sync.dma_start(c_all[:], c_ext[:, :, :])
            wk_sb = [cp.tile([128, D], BF16, tag=f"wk{k}", name=f"wk_sb{k}") for k in range(DK)]
            wv_sb = [cp.tile([128, D], BF16, tag=f"wv{k}", name=f"wv_sb{k}") for k in range(DK)]
            wq_sb = [cp.tile([128, D], BF16, tag=f"wq{k}", name=f"wq_sb{k}") for k in range(DK)]
            for k in range(DK):
                nc.scalar.dma_start(wk_sb[k][:], wk_ext.ap()[k])
            for k in range(DK):
                nc.scalar.dma_start(wv_sb[k][:], wv_ext.ap()[k])
            for k in range(DK):
                nc.scalar.dma_start(wq_sb[k][:], wq_ext.ap()[k])
            bias_sb = cp.tile([65, D], BF16)
            nc.scalar.dma_start(bias_sb[0:1, :], kb_ext[:, :])
            nc.scalar.dma_start(bias_sb[32:33, :], vb_ext[:, :])
            nc.scalar.dma_start(bias_sb[64:65, :], qb_ext[:, :])
            kb_sb, vb_sb, qb_sb = bias_sb[0:1, :], bias_sb[32:33, :], bias_sb[64:65, :]
            hwt_sb = cp.tile([128, DK, C7, C7], F32)
            nc.scalar.dma_start(hwt_sb[:], hwt_ext[:, :, :, :])
            hb_sb = cp.tile([1, C7], F32)
            nc.scalar.dma_start(hb_sb[:], hb_ext[:, :])
            eye_f = cp.tile([ncp, C7], F32)
            nc.scalar.dma_start(eye_f[:], eye_ext[:, :])

            p_all = cp.tile([128, rt, C7], BF16)
            a_full = cp.tile([128, rt, C7], F32)

            ones_bf = cp.tile([65, 128], BF16)
            nc.gpsimd.memset(ones_bf[:], 1.0)
            one_f = cp.tile([1, 1], F32)
            nc.gpsimd.memset(one_f[:], 1.0)

            # identity matrices (via iota + is_equal)
            iota_p = cp.tile([128, 1], I32)
            nc.gpsimd.iota(iota_p[:], [[0, 1]], channel_multiplier=1)
            iota_pf = cp.tile([128, 1], F32)
            nc.vector.tensor_copy(iota_pf[:], iota_p[:])
            iota_f = cp.tile([128, 128], I32)
            nc.gpsimd.iota(iota_f[:], [[1, 128]], channel_multiplier=0)
            iota_ff = cp.tile([128, 128], F32)
            nc.vector.tensor_copy(iota_ff[:], iota_f[:])
            ident_f = cp.tile([128, 128], F32)
            nc.vector.tensor_scalar(out=ident_f[:], in0=iota_ff[:], scalar1=iota_pf[:],
                                    scalar2=None, op0=ALU.is_equal)
            ident_bf = cp.tile([128, 128], BF16)
            nc.vector.tensor_copy(ident_bf[:], ident_f[:])

            # collective + spill dram buffers
            ms_in = dram.tile([C7, 1 + D], F32)
            ms_out = dram.tile([ncp, 1 + D], F32, addr_space="Shared")
            gm_d = dram.tile([1, C7], F32)
            red_in = dram.tile([C7, 1 + D], F32)
            red_out = dram.tile([C7, 1 + D], F32, addr_space="Shared")
            k1t_dram = dram.tile([rt, 128, DK, 128], BF16)

            # ---------------- local column max of c ----------------
            cmax = cp.tile([128, C7], F32)
            nc.vector.tensor_reduce(out=cmax[:], in_=c_all[:].rearrange("p t j -> p j t"),
                                    axis=mybir.AxisListType.X, op=ALU.max)
            ctp = ps_sm.tile([C7, 128], F32, tag="sm")
            nc.tensor.transpose(ctp[:], cmax[:], ident_f[:])
            mloc = cp.tile([C7, 1], F32)
            nc.vector.tensor_reduce(out=mloc[:], in_=ctp[:], axis=mybir.AxisListType.X,
                                    op=ALU.max)
            mltp = ps_sm.tile([C7, 128], F32, tag="sm")
            nc.tensor.transpose(mltp[0:1, 0:C7], mloc[:], ident_f[0:C7, 0:C7])
            ml1 = cp.tile([1, C7], F32)
            nc.vector.tensor_copy(ml1[:], mltp[0:1, 0:C7])
            mg128 = cp.tile([128, C7], F32)
            nc.gpsimd.partition_broadcast(mg128[:], ml1[:])

            # ---------------- helpers ----------------
            def rsqrt_cols(vv, y, t1, t2, p, w):
                nc.vector.tensor_scalar(out=y[:p, :w].bitcast(I32),
                                        in0=vv[:p, :w].bitcast(I32),
                                        scalar1=1, scalar2=-1,
                                        op0=ALU.logical_shift_right,
                                        op1=ALU.bitwise_xor)
                nc.vector.tensor_scalar(out=y[:p, :w].bitcast(I32),
                                        in0=y[:p, :w].bitcast(I32),
                                        scalar1=0x5F3759E0, scalar2=None, op0=ALU.add)
                for _ in range(2):
                    nc.vector.tensor_tensor(out=t1[:p, :w], in0=y[:p, :w],
                                            in1=y[:p, :w], op=ALU.mult)
                    nc.vector.tensor_tensor(out=t2[:p, :w], in0=t1[:p, :w],
                                            in1=vv[:p, :w], op=ALU.mult)
                    nc.vector.tensor_scalar(out=t2[:p, :w], in0=t2[:p, :w], scalar1=-0.5,
                                            scalar2=1.5, op0=ALU.mult, op1=ALU.add)
                    nc.vector.tensor_tensor(out=y[:p, :w], in0=y[:p, :w],
                                            in1=t2[:p, :w], op=ALU.mult)

            def ln_gelu2(hA, hB, outA, outB, p, tagp):
                """gelu((h - mean)/sqrt(var+eps)) over two [p, 512] psum halves."""
                bns = smp.tile([128, 12], F32, tag=f"bns{tagp}")
                mv = smp.tile([128, 2], F32, tag=f"mv{tagp}")
                vv = smp.tile([128, 1], F32, tag=f"vv{tagp}")
                y = smp.tile([128, 1], F32, tag=f"y{tagp}")
                t1 = smp.tile([128, 1], F32, tag=f"t1{tagp}")
                t2 = smp.tile([128, 1], F32, tag=f"t2{tagp}")
                nb = smp.tile([128, 1], F32, tag=f"nb{tagp}")
                nc.vector.bn_stats(bns[:p, 0:6], hA)
                nc.vector.bn_stats(bns[:p, 6:12], hB)
                nc.vector.bn_aggr(mv[:p, :], bns[:p, :])
                nc.vector.tensor_scalar(out=vv[:p, :], in0=mv[:p, 1:2], scalar1=EPS,
                                        scalar2=None, op0=ALU.add)
                rsqrt_cols(vv, y, t1, t2, p, 1)
                nc.vector.scalar_tensor_tensor(out=nb[:p, :], in0=mv[:p, 0:1],
                                               scalar=-1.0, in1=y[:p, :],
                                               op0=ALU.mult, op1=ALU.mult)
                nc.scalar.activation(outA, hA, AF.Gelu, bias=nb[:p, :], scale=y[:p, :])
                nc.scalar.activation(outB, hB, AF.Gelu, bias=nb[:p, :], scale=y[:p, :])

            def chain_mms(tgt, lhs_fn, w_sb, b_sb, p):
                for oc in range(2):
                    sl = slice(oc * 512, (oc + 1) * 512)
                    for k in range(DK):
                        nc.tensor.matmul(tgt(oc), lhs_fn(k), w_sb[k][:, sl],
                                         start=(k == 0), stop=False)
                    bp_ = b_sb.base_partition()
                    nc.tensor.matmul(tgt(oc), ones_bf[bp_:bp_ + 1, :p],
                                     b_sb[:1, sl], start=False, stop=True)

            # ================= pass A: K chains + local selection =============
            msel_ps = ps_acc.tile([C7, D], F32, tag="acc")
            for t in range(rt):
                xt_t = iop.tile([128, DK, 128], BF16, tag="xt")
                nc.gpsimd.dma_start(xt_t[:], xt_ext.ap()[t])
                hh = [ps_h.tile([128, 512], F32, tag="h", name=f"hk{t}_{o}")
                      for o in range(2)]
                chain_mms(lambda oc: hh[oc][:, :], lambda k: xt_t[:, k, :],
                          wk_sb, kb_sb, 128)
                k1 = ap_.tile([128, D], BF16, tag="k1")
                ln_gelu2(hh[0][:, :], hh[1][:, :], k1[:, 0:512], k1[:, 512:1024],
                         128, "m")
                pvr = vr_last if t == rt - 1 else 128
                oh = smp.tile([128, C7], BF16, tag="oh")
                nc.vector.tensor_tensor(out=oh[:], in0=c_all[:, t, :], in1=mg128[:],
                                        op=ALU.is_equal)
                for oc in range(2):
                    sl = slice(oc * 512, (oc + 1) * 512)
                    nc.tensor.matmul(msel_ps[:, sl], oh[:pvr, :], k1[:pvr, sl],
                                     start=(t == 0), stop=(t == rt - 1))
                k1t = ap_.tile([128, DK, 128], BF16, tag="k1t")
                nc.sync.dma_start(k1t[:], k1[:], transpose=True)
                nc.gpsimd.dma_start(k1t_dram[t], k1t[:])

            # ======== m_sel gather + winner selection + q_max chain ==========
            msc = cp.tile([C7, 1 + D], F32)
            nc.vector.tensor_copy(msc[:, 0:1], mloc[:])
            nc.vector.tensor_copy(msc[:, 1:1 + D], msel_ps[:])
            nc.sync.dma_start(ms_in[:], msc[:])
            nc.gpsimd.collective_compute(
                "AllGather", ALU.bypass, replica_groups=rg,
                ins=[ms_in[:].opt()], outs=[ms_out[:].opt()])
            g_sb = cp.tile([ncp, 1 + D], F32)
            nc.sync.dma_start(g_sb[:], ms_out[:])
            mv78 = cp.tile([C7, n_cores], F32)
            nc.sync.dma_start(mv78[:],
                              ms_out[:].rearrange("(s j) o -> j s o", j=C7)[:, :, 0])
            gmax = cp.tile([C7, 1], F32)
            nc.vector.tensor_reduce(out=gmax[:], in_=mv78[:],
                                    axis=mybir.AxisListType.X, op=ALU.max)
            nc.sync.dma_start(gm_d[:], gmax[:])
            gmrep = cp.tile([ncp, 1], F32)
            nc.sync.dma_start(gmrep[:], gm_d[0:1, :].partition_broadcast(n_cores))
            mv56 = cp.tile([ncp, 1], F32)
            nc.sync.dma_start(mv56[:], ms_out[:, 0:1])
            masks = cp.tile([ncp, 1], F32)
            nc.vector.tensor_scalar(out=masks[:], in0=mv56[:], scalar1=gmrep[:],
                                    scalar2=None, op0=ALU.is_equal)
            sel_bf = cp.tile([ncp, C7], BF16)
            nc.vector.tensor_scalar(out=sel_bf[:], in0=eye_f[:], scalar1=masks[:],
                                    scalar2=None, op0=ALU.mult)
            g_bf = cp.tile([ncp, D], BF16)
            nc.vector.tensor_copy(g_bf[:], g_sb[:, 1:1 + D])
            mselT_ps = ps_sm.tile([128, C7 * DK], F32, tag="sm")
            for ob in range(DK):
                nc.tensor.matmul(mselT_ps[:, ob * C7:(ob + 1) * C7],
                                 g_bf[:, ob * 128:(ob + 1) * 128], sel_bf[:],
                                 start=True, stop=True)
            mselT = cp.tile([128, DK, C7], BF16)
            nc.vector.tensor_copy(mselT[:], mselT_ps[:])

            hq_ps = ps_acc.tile([C7, D], F32, tag="acc")
            chain_mms(lambda oc: hq_ps[:C7, oc * 512:(oc + 1) * 512],
                      lambda k: mselT[:, k, :], wq_sb, qb_sb, C7)
            qmax = cp.tile([C7, D], BF16)
            ln_gelu2(hq_ps[:C7, 0:512], hq_ps[:C7, 512:1024],
                     qmax[:, 0:512], qmax[:, 512:1024], C7, "q")
            qmaxT = cp.tile([128, DK, C7], BF16)
            for k in range(DK):
                tpb = ps_sm.tile([128, 128], BF16, tag="sm")
                nc.tensor.transpose(tpb[:, 0:C7], qmax[:, k * 128:(k + 1) * 128],
                                    ident_bf[0:C7, 0:C7])
                nc.vector.tensor_copy(qmaxT[:, k, :], tpb[:, 0:C7])

            # ======== pass B: V/Q chains fused with logits/exp/Bp ============
            bp_ps = ps_acc.tile([C7, D], F32, tag="acc")
            cs_acc = cp.tile([C7, 2], F32)
            nc.gpsimd.memset(cs_acc[:], 0.0)
            qt_tiles = {}
            v_tiles = {}
            LAGB = 10

            def emit_vq(t):
                k1t_t = iop.tile([128, DK, 128], BF16, tag="k1tl")
                nc.gpsimd.dma_start(k1t_t[:], k1t_dram[t])
                hv = [ps_h.tile([128, 512], F32, tag="h", name=f"hv{t}_{o}")
                      for o in range(2)]
                chain_mms(lambda oc: hv[oc][:, :], lambda k: k1t_t[:, k, :],
                          wv_sb, vb_sb, 128)
                v_t = vp.tile([128, D], BF16, tag="v")
                ln_gelu2(hv[0][:, :], hv[1][:, :], v_t[:, 0:512], v_t[:, 512:1024],
                         128, "m")
                hq = [ps_h.tile([128, 512], F32, tag="h", name=f"hqv{t}_{o}")
                      for o in range(2)]
                chain_mms(lambda oc: hq[oc][:, :], lambda k: k1t_t[:, k, :],
                          wq_sb, qb_sb, 128)
                q1 = ap_.tile([128, D], BF16, tag="q1")
                ln_gelu2(hq[0][:, :], hq[1][:, :], q1[:, 0:512], q1[:, 512:1024],
                         128, "m")
                qt = qtp.tile([128, DK, 128], BF16, tag="qt")
                nc.sync.dma_start(qt[:], q1[:], transpose=True)
                qt_tiles[t] = qt
                v_tiles[t] = v_t

            def emit_att(t):
                qt = qt_tiles.pop(t)
                v_t = v_tiles.pop(t)
                pvr = vr_last if t == rt - 1 else 128
                lg = ps_sm.tile([C7, 128], F32, tag="sm", name=f"lg{t}")
                for k in range(DK):
                    nc.tensor.matmul(lg[:C7, :], qmaxT[:, k, :], qt[:, k, :],
                                     start=(k == 0), stop=(k == DK - 1))
                pt_t = smp.tile([C7, 128], BF16, tag="ptt")
                nc.scalar.activation(pt_t[:], lg[:], AF.Exp,
                                     scale=float(1.0 / np.sqrt(D)))
                nc.vector.tensor_reduce(out=cs_acc[:, 1:2], in_=pt_t[:, 0:pvr],
                                        axis=mybir.AxisListType.X, op=ALU.add)
                nc.vector.tensor_tensor(out=cs_acc[:, 0:1], in0=cs_acc[:, 0:1],
                                        in1=cs_acc[:, 1:2], op=ALU.add)
                ptp = ps_sm.tile([128, 128], BF16, tag="sm", name=f"ptp{t}")
                nc.tensor.transpose(ptp[:, 0:C7], pt_t[:C7, :], ident_bf[0:C7, 0:C7])
                nc.vector.tensor_copy(p_all[:, t, :], ptp[:, 0:C7])
                for oc in range(2):
                    sl = slice(oc * 512, (oc + 1) * 512)
                    nc.tensor.matmul(bp_ps[:, sl], p_all[:pvr, t, :], v_t[:pvr, sl],
                                     start=(t == 0), stop=(t == rt - 1))

            # emit attention stages in groups of G so the exp activations run
            # back-to-back (2 ScalarE table loads per group, not per tile)
            G = 8
            ready = []
            done_att = 0
            for t in range(rt):
                emit_vq(t)
                if t >= LAGB:
                    ready.append(t - LAGB)
                if len(ready) >= G:
                    for u in ready:
                        emit_att(u)
                    done_att += len(ready)
                    ready = []
            for u in ready:
                emit_att(u)
            for t in range(rt - LAGB, rt):
                emit_att(t)

            # ---------------- final reduce: [colsum | Bp] ----------------
            red_sb = cp.tile([C7, 1 + D], F32)
            nc.vector.tensor_copy(red_sb[:, 0:1], cs_acc[:, 0:1])
            nc.vector.tensor_copy(red_sb[:, 1:1 + D], bp_ps[:])
            nc.sync.dma_start(red_in[:], red_sb[:])
            nc.gpsimd.collective_compute(
                "AllReduce", ALU.add, replica_groups=rg,
                ins=[red_in[:].opt()], outs=[red_out[:].opt()])
            red2 = red_sb
            nc.sync.dma_start(red2[:], red_out[:])

            # B = Bp / denom (per class)
            rden = cp.tile([C7, 1], F32)
            nc.vector.reciprocal(rden[:], red2[:, 0:1])
            b_sb = cp.tile([C7, D], F32)
            nc.vector.tensor_scalar(out=b_sb[:], in0=red2[:, 1:1 + D], scalar1=rden[:],
                                    scalar2=None, op0=ALU.mult)
            nc.sync.dma_start(b_ext[:, :], b_sb[:])

            # A = P / denom: pre-copy P (overlaps the AllReduce), scale in place
            nc.vector.tensor_copy(a_full[:], p_all[:])
            den1 = cp.tile([1, C7], F32)
            nc.sync.dma_start(den1[:], red_out[:].rearrange("j o -> o j")[0:1, :])
            rden1 = cp.tile([1, C7], F32)
            nc.vector.reciprocal(rden1[:], den1[:])
            rb128 = cp.tile([128, C7], F32)
            nc.gpsimd.partition_broadcast(rb128[:], rden1[:])
            for t in range(rt):
                nc.vector.tensor_tensor(out=a_full[:, t, :], in0=a_full[:, t, :],
                                        in1=rb128[:], op=ALU.mult)
            nc.sync.dma_start(a_ext[:, :, :], a_full[:])

            # C = einsum(B, head_w) + head_b
            btf = cp.tile([128, DK, C7], F32)
            for k in range(DK):
                tp = ps_sm.tile([128, 128], F32, tag="sm")
                nc.tensor.transpose(tp[:, 0:C7], b_sb[:, k * 128:(k + 1) * 128],
                                    ident_f[0:C7, 0:C7])
                nc.vector.tensor_copy(btf[:, k, :], tp[:, 0:C7])
            c_ps = ps_sm.tile([1, C7], F32, tag="sm")
            first = True
            for kb in range(DK):
                for i in range(C7):
                    nc.tensor.matmul(c_ps[:], btf[:, kb, i:i + 1], hwt_sb[:, kb, i, :],
                                     start=first, stop=False)
                    first = False
            nc.tensor.matmul(c_ps[:], one_f[:], hb_sb[:], start=False, stop=True)
            c_sb = cp.tile([1, C7], F32)
            nc.vector.tensor_copy(c_sb[:], c_ps[:])
            nc.sync.dma_start(co_ext[:, :], c_sb[:])

    nc.compile()
    return nc


# --------------------------------------------------------------------------
# host-side prep / unshard
# --------------------------------------------------------------------------

def _prep_core_inputs(shard_x, shard_c, rt):
    """shard_x [rs, 1024] f32, shard_c [rs, 7] f32 -> device layouts."""
    import ml_dtypes
    rs = shard_x.shape[0]
    rpad = rt * 128
    xp = np.zeros((rpad, D), dtype=np.float32)
    xp[:rs] = shard_x
    # [t, p(i_in_tile), k(i_tile), r] ; value = x[t*128+r, k*128+p]
    xt = np.ascontiguousarray(
        xp.reshape(rt, 128, DK, 128).transpose(0, 3, 2, 1)
    ).astype(ml_dtypes.bfloat16)
    cp_ = np.full((rpad, C7), NEG_BIG, dtype=np.float32)
    cp_[:rs] = shard_c
    call = np.ascontiguousarray(cp_.reshape(rt, 128, C7).transpose(1, 0, 2))
    return xt, call


def _prep_shared_inputs(inp):
    import ml_dtypes
    bf = ml_dtypes.bfloat16
    out = {}
    for nm, key in (("wk", "key_w"), ("wv", "value_w"), ("wq", "query_w")):
        w = np.asarray(inp[key], dtype=np.float32)
        out[nm] = np.ascontiguousarray(w.T.reshape(DK, 128, D)).astype(bf)
    for nm, key in (("kb", "key_b"), ("vb", "value_b"), ("qb", "query_b")):
        out[nm] = np.asarray(inp[key], dtype=np.float32).reshape(1, D).astype(bf)
    hw = np.asarray(inp["head_w"], dtype=np.float32)  # [7, 7, 1024]
    out["hwt"] = np.ascontiguousarray(
        hw.transpose(2, 1, 0).reshape(DK, 128, C7, C7).transpose(1, 0, 2, 3)
    )
    out["hb"] = np.asarray(inp["head_b"], dtype=np.float32).reshape(1, C7)
    out["eye"] = np.tile(np.eye(C7, dtype=np.float32), (N_CORES, 1))
    return out


def kernel(**inputs) -> tuple:
    feats = np.asarray(inputs["features"], dtype=np.float32)
    c = np.asarray(inputs["c"], dtype=np.float32)
    n = feats.shape[0]
    assert n % N_CORES == 0
    rs = n // N_CORES
    rt = (rs + 127) // 128

    key = (rs, N_CORES)
    if key not in _BUILD_CACHE:
        _BUILD_CACHE[key] = build_graph(rs, N_CORES)
    nc = _BUILD_CACHE[key]

    shared = _prep_shared_inputs(inputs)
    in_maps = []
    for s in range(N_CORES):
        xt, call = _prep_core_inputs(feats[s * rs:(s + 1) * rs],
                                     c[s * rs:(s + 1) * rs], rt)
        m = {"xt": xt, "call": call}
        m.update(shared)
        in_maps.append(m)

    res = run_bass_kernel_spmd(nc, in_maps, core_ids=list(range(N_CORES)))

    a_parts = []
    for s in range(N_CORES):
        a_po = res.results[s]["A_out"]                      # [128, rt, 7]
        a_parts.append(a_po.transpose(1, 0, 2).reshape(rt * 128, C7)[:rs])
    A = np.concatenate(a_parts, axis=0).astype(np.float32)
    B = res.results[0]["B_out"].reshape(1, C7, D).astype(np.float32)
    Cc = res.results[0]["C_out"].reshape(1, C7).astype(np.float32)
    return (Cc, A, B)


# revision 32
# speedup vs baseline: 1.0101x; 1.0101x over previous
"""Trainium2 Bass kernel for nn_AttDual: dual-attention MIL pooling head.

Computation (see reference):
  K = chain(features, key_*)    ; chain = Linear -> LayerNorm -> GELU(erf)
  V = chain(K, value_*)
  Q = chain(K, query_*)
  top_idx = argmax(c, axis=0)   ; q_max = chain(K[top_idx], query_*)  (== Q[top_idx])
  A = softmax(Q @ q_max.T / 32, axis=0)   (column softmax over all N)
  B = A.T @ V ;  C = einsum('ik,oik->o', B, head_w) + head_b
Returns (C [1,7], A [N,7], B [1,7,1024]).

Distribution: data-parallel over N across 8 NeuronCores. Collectives:
one AllGather of [local-argmax value | selected K row] per class (each core
then picks the winners locally), and one AllReduce of [colsum | B].

Structure: pass A streams features and computes K tiles (K^T spilled to
DRAM) plus the local-argmax selection; pass B streams K^T back and fuses
the V/Q chains with logits -> exp -> B-accumulation per tile, lagged a few
tiles so the q_max chain hides under the head of pass B.

Matmuls in bf16 with f32 PSUM accumulation. LayerNorm stats via bn_stats;
rsqrt via DVE bit-trick + 2 Newton steps (keeps ScalarE on the Gelu table
through both passes; one switch to the exp table for the softmax).
"""
import numpy as np

import concourse.bacc as bacc
import concourse.tile as tile
import concourse.mybir as mybir
from concourse.bass_utils import run_bass_kernel_spmd

F32 = mybir.dt.float32
I32 = mybir.dt.int32
BF16 = mybir.dt.bfloat16
AF = mybir.ActivationFunctionType
ALU = mybir.AluOpType

N_CORES = 8
D = 1024
DK = 8          # d / 128
C7 = 7
EPS = 1e-5
NEG_BIG = -1.0e30

_BUILD_CACHE: dict = {}


def build_graph(rs: int, n_cores: int = N_CORES):
    """rs = rows per core (real). Tiles of 128 rows; last tile partial."""
    rt = (rs + 127) // 128
    vr_last = rs - (rt - 1) * 128
    ncp = n_cores * C7

    nc = bacc.Bacc("TRN2", target_bir_lowering=False, debug=False,
                   num_devices=n_cores)

    xt_ext = nc.declare_dram_parameter("xt", [rt, 128, DK, 128], BF16, isOutput=False)
    c_ext = nc.declare_dram_parameter("call", [128, rt, C7], F32, isOutput=False)
    wk_ext = nc.declare_dram_parameter("wk", [DK, 128, D], BF16, isOutput=False)
    wv_ext = nc.declare_dram_parameter("wv", [DK, 128, D], BF16, isOutput=False)
    wq_ext = nc.declare_dram_parameter("wq", [DK, 128, D], BF16, isOutput=False)
    kb_ext = nc.declare_dram_parameter("kb", [1, D], BF16, isOutput=False)
    vb_ext = nc.declare_dram_parameter("vb", [1, D], BF16, isOutput=False)
    qb_ext = nc.declare_dram_parameter("qb", [1, D], BF16, isOutput=False)
    hwt_ext = nc.declare_dram_parameter("hwt", [128, DK, C7, C7], F32, isOutput=False)
    hb_ext = nc.declare_dram_parameter("hb", [1, C7], F32, isOutput=False)
    eye_ext = nc.declare_dram_parameter("eye", [ncp, C7], F32, isOutput=False)

    a_ext = nc.declare_dram_parameter("A_out", [128, rt, C7], F32, isOutput=True)
    b_ext = nc.declare_dram_parameter("B_out", [C7, D], F32, isOutput=True)
    co_ext = nc.declare_dram_parameter("C_out", [1, C7], F32, isOutput=True)

    rg = [list(range(n_cores))]

    with tile.TileContext(nc) as tc:
        with (
            tc.tile_pool(name="const", bufs=1) as cp,
            tc.tile_pool(name="io", bufs=3) as iop,
            tc.tile_pool(name="act", bufs=2) as ap_,
            tc.tile_pool(name="qtp", bufs=15) as qtp,
            tc.tile_pool(name="vp", bufs=15) as vp,
            tc.tile_pool(name="sm", bufs=2) as smp,
            tc.tile_pool(name="ps_h", bufs=4, space="PSUM") as ps_h,
            tc.tile_pool(name="ps_acc", bufs=1, space="PSUM") as ps_acc,
            tc.tile_pool(name="ps_sm", bufs=2, space="PSUM") as ps_sm,
            tc.tile_pool(name="dram", bufs=1, space="DRAM") as dram,
        ):
            # ---------------- constants / pinned ----------------
            c_all = cp.tile([128, rt, C7], F32)
            nc.sync.dma_start(c_all[:], c_ext[:, :, :])
            wk_sb = [cp.tile([128, D], BF16, tag=f"wk{k}", name=f"wk_sb{k}") for k in range(DK)]
            wv_sb = [cp.tile([128, D], BF16, tag=f"wv{k}", name=f"wv_sb{k}") for k in range(DK)]
            wq_sb = [cp.tile([128, D], BF16, tag=f"wq{k}", name=f"wq_sb{k}") for k in range(DK)]
            for k in range(DK):
                nc.scalar.dma_start(wk_sb[k][:], wk_ext.ap()[k])
            for k in range(DK):
                nc.scalar.dma_start(wv_sb[k][:], wv_ext.ap()[k])
            for k in range(DK):
                nc.scalar.dma_start(wq_sb[k][:], wq_ext.ap()[k])
            bias_sb = cp.tile([65, D], BF16)
            nc.scalar.dma_start(bias_sb[0:1, :], kb_ext[:, :])
            nc.scalar.dma_start(bias_sb[32:33, :], vb_ext[:, :])
            nc.scalar.dma_start(bias_sb[64:65, :], qb_ext[:, :])
            kb_sb, vb_sb, qb_sb = bias_sb[0:1, :], bias_sb[32:33, :], bias_sb[64:65, :]
            hwt_sb = cp.tile([128, DK, C7, C7], F32)
            nc.scalar.dma_start(hwt_sb[:], hwt_ext[:, :, :, :])
            hb_sb = cp.tile([1, C7], F32)
            nc.scalar.dma_start(hb_sb[:], hb_ext[:, :])
            eye_f = cp.tile([ncp, C7], F32)
            nc.scalar.dma_start(eye_f[:], eye_ext[:, :])

            p_all = cp.tile([128, rt, C7], BF16)
            a_full = cp.tile([128, rt, C7], F32)

            ones_bf = cp.tile([65, 128], BF16)
            nc.gpsimd.memset(ones_bf[:], 1.0)
            one_f = cp.tile([1, 1], F32)
            nc.gpsimd.memset(one_f[:], 1.0)

            # identity matrices (via iota + is_equal)
            iota_p = cp.tile([128, 1], I32)
            nc.gpsimd.iota(iota_p[:], [[0, 1]], channel_multiplier=1)
            iota_pf = cp.tile([128, 1], F32)
            nc.vector.tensor_copy(iota_pf[:], iota_p[:])
            iota_f = cp.tile([128, 128], I32)
            nc.gpsimd.iota(iota_f[:], [[1, 128]], channel_multiplier=0)
            iota_ff = cp.tile([128, 128], F32)
            nc.vector.tensor_copy(iota_ff[:], iota_f[:])
            ident_f = cp.tile([128, 128], F32)
            nc.vector.tensor_scalar(out=ident_f[:], in0=iota_ff[:], scalar1=iota_pf[:],
                                    scalar2=None, op0=ALU.is_equal)
            ident_bf = cp.tile([128, 128], BF16)
            nc.vector.tensor_copy(ident_bf[:], ident_f[:])

            # collective + spill dram buffers
            ms_in = dram.tile([C7, 1 + D], F32)
            ms_out = dram.tile([ncp, 1 + D], F32, addr_space="Shared")
            gm_d = dram.tile([1, C7], F32)
            red_in = dram.tile([C7, 1 + D], F32)
            red_out = dram.tile([C7, 1 + D], F32, addr_space="Shared")
            k1t_dram = dram.tile([rt, 128, DK, 128], BF16)

            # ---------------- local column max of c ----------------
            cmax = cp.tile([128, C7], F32)
            nc.vector.tensor_reduce(out=cmax[:], in_=c_all[:].rearrange("p t j -> p j t"),
                                    axis=mybir.AxisListType.X, op=ALU.max)
            ctp = ps_sm.tile([C7, 128], F32, tag="sm")
            nc.tensor.transpose(ctp[:], cmax[:], ident_f[:])
            mloc = cp.tile([C7, 1], F32)
            nc.vector.tensor_reduce(out=mloc[:], in_=ctp[:], axis=mybir.AxisListType.X,
                                    op=ALU.max)
            mltp = ps_sm.tile([C7, 128], F32, tag="sm")
            nc.tensor.transpose(mltp[0:1, 0:C7], mloc[:], ident_f[0:C7, 0:C7])
            ml1 = cp.tile([1, C7], F32)
            nc.vector.tensor_copy(ml1[:], mltp[0:1, 0:C7])
            mg128 = cp.tile([128, C7], F32)
            nc.gpsimd.partition_broadcast(mg128[:], ml1[:])

            # ---------------- helpers ----------------
            def rsqrt_cols(vv, y, t1, t2, p, w):
                nc.vector.tensor_scalar(out=y[:p, :w].bitcast(I32),
                                        in0=vv[:p, :w].bitcast(I32),
                                        scalar1=1, scalar2=-1,
                                        op0=ALU.logical_shift_right,
                                        op1=ALU.bitwise_xor)
                nc.vector.tensor_scalar(out=y[:p, :w].bitcast(I32),
                                        in0=y[:p, :w].bitcast(I32),
                                        scalar1=0x5F3759E0, scalar2=None, op0=ALU.add)
                for _ in range(2):
                    nc.vector.tensor_tensor(out=t1[:p, :w], in0=y[:p, :w],
                                            in1=y[:p, :w], op=ALU.mult)
                    nc.vector.tensor_tensor(out=t2[:p, :w], in0=t1[:p, :w],
                                            in1=vv[:p, :w], op=ALU.mult)
                    nc.vector.tensor_scalar(out=t2[:p, :w], in0=t2[:p, :w], scalar1=-0.5,
                                            scalar2=1.5, op0=ALU.mult, op1=ALU.add)
                    nc.vector.tensor_tensor(out=y[:p, :w], in0=y[:p, :w],
                                            in1=t2[:p, :w], op=ALU.mult)

            def ln_gelu2(hA, hB, outA, outB, p, tagp):
                """gelu((h - mean)/sqrt(var+eps)) over two [p, 512] psum halves."""
                bns = smp.tile([128, 12], F32, tag=f"bns{tagp}")
                mv = smp.tile([128, 2], F32, tag=f"mv{tagp}")
                vv = smp.tile([128, 1], F32, tag=f"vv{tagp}")
                y = smp.tile([128, 1], F32, tag=f"y{tagp}")
                t1 = smp.tile([128, 1], F32, tag=f"t1{tagp}")
                t2 = smp.tile([128, 1], F32, tag=f"t2{tagp}")
                nb = smp.tile([128, 1], F32, tag=f"nb{tagp}")
                nc.vector.bn_stats(bns[:p, 0:6], hA)
                nc.vector.bn_stats(bns[:p, 6:12], hB)
                nc.vector.bn_aggr(mv[:p, :], bns[:p, :])
                nc.vector.tensor_scalar(out=vv[:p, :], in0=mv[:p, 1:2], scalar1=EPS,
                                        scalar2=None, op0=ALU.add)
                rsqrt_cols(vv, y, t1, t2, p, 1)
                nc.vector.scalar_tensor_tensor(out=nb[:p, :], in0=mv[:p, 0:1],
                                               scalar=-1.0, in1=y[:p, :],
                                               op0=ALU.mult, op1=ALU.mult)
                nc.scalar.activation(outA, hA, AF.Gelu, bias=nb[:p, :], scale=y[:p, :])
                nc.scalar.activation(outB, hB, AF.Gelu, bias=nb[:p, :], scale=y[:p, :])

            def chain_mms(tgt, lhs_fn, w_sb, b_sb, p):
                for oc in range(2):
                    sl = slice(oc * 512, (oc + 1) * 512)
                    for k in range(DK):
                        nc.tensor.matmul(tgt(oc), lhs_fn(k), w_sb[k][:, sl],
                                         start=(k == 0), stop=False)
                    bp_ = b_sb.base_partition()
                    nc.tensor.matmul(tgt(oc), ones_bf[bp_:bp_ + 1, :p],
                                     b_sb[:1, sl], start=False, stop=True)

            # ================= pass A: K chains + local selection =============
            msel_ps = ps_acc.tile([C7, D], F32, tag="acc")
            for t in range(rt):
                xt_t = iop.tile([128, DK, 128], BF16, tag="xt")
                nc.gpsimd.dma_start(xt_t[:], xt_ext.ap()[t])
                hh = [ps_h.tile([128, 512], F32, tag="h", name=f"hk{t}_{o}")
                      for o in range(2)]
                chain_mms(lambda oc: hh[oc][:, :], lambda k: xt_t[:, k, :],
                          wk_sb, kb_sb, 128)
                k1 = ap_.tile([128, D], BF16, tag="k1")
                ln_gelu2(hh[0][:, :], hh[1][:, :], k1[:, 0:512], k1[:, 512:1024],
                         128, "m")
                pvr = vr_last if t == rt - 1 else 128
                oh = smp.tile([128, C7], BF16, tag="oh")
                nc.vector.tensor_tensor(out=oh[:], in0=c_all[:, t, :], in1=mg128[:],
                                        op=ALU.is_equal)
                for oc in range(2):
                    sl = slice(oc * 512, (oc + 1) * 512)
                    nc.tensor.matmul(msel_ps[:, sl], oh[:pvr, :], k1[:pvr, sl],
                                     start=(t == 0), stop=(t == rt - 1))
                k1t = ap_.tile([128, DK, 128], BF16, tag="k1t")
                nc.sync.dma_start(k1t[:], k1[:], transpose=True)
                nc.gpsimd.dma_start(k1t_dram[t], k1t[:])

            # ======== m_sel gather + winner selection + q_max chain ==========
            msc = cp.tile([C7, 1 + D], F32)
            nc.vector.tensor_copy(msc[:, 0:1], mloc[:])
            nc.vector.tensor_copy(msc[:, 1:1 + D], msel_ps[:])
            nc.sync.dma_start(ms_in[:], msc[:])
            nc.gpsimd.collective_compute(
                "AllGather", ALU.bypass, replica_groups=rg,
                ins=[ms_in[:].opt()], outs=[ms_out[:].opt()])
            g_sb = cp.tile([ncp, 1 + D], F32)
            nc.sync.dma_start(g_sb[:], ms_out[:])
            mv78 = cp.tile([C7, n_cores], F32)
            nc.sync.dma_start(mv78[:],
                              ms_out[:].rearrange("(s j) o -> j s o", j=C7)[:, :, 0])
            gmax = cp.tile([C7, 1], F32)
            nc.vector.tensor_reduce(out=gmax[:], in_=mv78[:],
                                    axis=mybir.AxisListType.X, op=ALU.max)
            nc.sync.dma_start(gm_d[:], gmax[:])
            gmrep = cp.tile([ncp, 1], F32)
            nc.sync.dma_start(gmrep[:], gm_d[0:1, :].partition_broadcast(n_cores))
            mv56 = cp.tile([ncp, 1], F32)
            nc.sync.dma_start(mv56[:], ms_out[:, 0:1])
            masks = cp.tile([ncp, 1], F32)
            nc.vector.tensor_scalar(out=masks[:], in0=mv56[:], scalar1=gmrep[:],
                                    scalar2=None, op0=ALU.is_equal)
            sel_bf = cp.tile([ncp, C7], BF16)
            nc.vector.tensor_scalar(out=sel_bf[:], in0=eye_f[:], scalar1=masks[:],
                                    scalar2=None, op0=ALU.mult)
            g_bf = cp.tile([ncp, D], BF16)
            nc.vector.tensor_copy(g_bf[:], g_sb[:, 1:1 + D])
            mselT_ps = ps_sm.tile([128, C7 * DK], F32, tag="sm")
            for ob in range(DK):
                nc.tensor.matmul(mselT_ps[:, ob * C7:(ob + 1) * C7],
                                 g_bf[:, ob * 128:(ob + 1) * 128], sel_bf[:],
                                 start=True, stop=True)
            mselT = cp.tile([128, DK, C7], BF16)
            nc.vector.tensor_copy(mselT[:], mselT_ps[:])

            hq_ps = ps_acc.tile([C7, D], F32, tag="acc")
            chain_mms(lambda oc: hq_ps[:C7, oc * 512:(oc + 1) * 512],
                      lambda k: mselT[:, k, :], wq_sb, qb_sb, C7)
            qmax = cp.tile([C7, D], BF16)
            ln_gelu2(hq_ps[:C7, 0:512], hq_ps[:C7, 512:1024],
                     qmax[:, 0:512], qmax[:, 512:1024], C7, "q")
            qmaxT = cp.tile([128, DK, C7], BF16)
            for k in range(DK):
                tpb = ps_sm.tile([128, 128], BF16, tag="sm")
                nc.tensor.transpose(tpb[:, 0:C7], qmax[:, k * 128:(k + 1) * 128],
                                    ident_bf[0:C7, 0:C7])
                nc.vector.tensor_copy(qmaxT[:, k, :], tpb[:, 0:C7])

            # ======== pass B: V/Q chains fused with logits/exp/Bp ============
            bp_ps = ps_acc.tile([C7, D], F32, tag="acc")
            cs_acc = cp.tile([C7, 2], F32)
            nc.gpsimd.memset(cs_acc[:], 0.0)
            qt_tiles = {}
            v_tiles = {}
            LAGB = 5

            def emit_vq(t):
                k1t_t = iop.tile([128, DK, 128], BF16, tag="k1tl")
                nc.gpsimd.dma_start(k1t_t[:], k1t_dram[t])
                hv = [ps_h.tile([128, 512], F32, tag="h", name=f"hv{t}_{o}")
                      for o in range(2)]
                chain_mms(lambda oc: hv[oc][:, :], lambda k: k1t_t[:, k, :],
                          wv_sb, vb_sb, 128)
                v_t = vp.tile([128, D], BF16, tag="v")
                ln_gelu2(hv[0][:, :], hv[1][:, :], v_t[:, 0:512], v_t[:, 512:1024],
                         128, "m")
                hq = [ps_h.tile([128, 512], F32, tag="h", name=f"hqv{t}_{o}")
                      for o in range(2)]
                chain_mms(lambda oc: hq[oc][:, :], lambda k: k1t_t[:, k, :],
                          wq_sb, qb_sb, 128)
                q1 = ap_.tile([128, D], BF16, tag="q1")
                ln_gelu2(hq[0][:, :], hq[1][:, :], q1[:, 0:512], q1[:, 512:1024],
                         128, "m")
                qt = qtp.tile([128, DK, 128], BF16, tag="qt")
                nc.sync.dma_start(qt[:], q1[:], transpose=True)
                qt_tiles[t] = qt
                v_tiles[t] = v_t

            def emit_att(t):
                qt = qt_tiles.pop(t)
                v_t = v_tiles.pop(t)
                pvr = vr_last if t == rt - 1 else 128
                lg = ps_sm.tile([C7, 128], F32, tag="sm", name=f"lg{t}")
                for k in range(DK):
                    nc.tensor.matmul(lg[:C7, :], qmaxT[:, k, :], qt[:, k, :],
                                     start=(k == 0), stop=(k == DK - 1))
                pt_t = smp.tile([C7, 128], BF16, tag="ptt")
                nc.scalar.activation(pt_t[:], lg[:], AF.Exp,
                                     scale=float(1.0 / np.sqrt(D)))
                nc.vector.tensor_reduce(out=cs_acc[:, 1:2], in_=pt_t[:, 0:pvr],
                                        axis=mybir.AxisListType.X, op=ALU.add)
                nc.vector.tensor_tensor(out=cs_acc[:, 0:1], in0=cs_acc[:, 0:1],
                                        in1=cs_acc[:, 1:2], op=ALU.add)
                ptp = ps_sm.tile([128, 128], BF16, tag="sm", name=f"ptp{t}")
                nc.tensor.transpose(ptp[:, 0:C7], pt_t[:C7, :], ident_bf[0:C7, 0:C7])
                nc.vector.tensor_copy(p_all[:, t, :], ptp[:, 0:C7])
                for oc in range(2):
                    sl = slice(oc * 512, (oc + 1) * 512)
                    nc.tensor.matmul(bp_ps[:, sl], p_all[:pvr, t, :], v_t[:pvr, sl],
                                     start=(t == 0), stop=(t == rt - 1))

            # emit attention stages in groups of G so the exp activations run
            # back-to-back (2 ScalarE table loads per group, not per tile)
            G = 8
            ready = []
            done_att = 0
            for t in range(rt):
                emit_vq(t)
                if t >= LAGB:
                    ready.append(t - LAGB)
                if len(ready) >= G:
                    for u in ready:
                        emit_att(u)
                    done_att += len(ready)
                    ready = []
            for u in ready:
                emit_att(u)
            for t in range(rt - LAGB, rt):
                emit_att(t)

            # ---------------- final reduce: [colsum | Bp] ----------------
            red_sb = cp.tile([C7, 1 + D], F32)
            nc.vector.tensor_copy(red_sb[:, 0:1], cs_acc[:, 0:1])
            nc.vector.tensor_copy(red_sb[:, 1:1 + D], bp_ps[:])
            nc.sync.dma_start(red_in[:], red_sb[:])
            nc.gpsimd.collective_compute(
                "AllReduce", ALU.add, replica_groups=rg,
                ins=[red_in[:].opt()], outs=[red_out[:].opt()])
            red2 = red_sb
            nc.sync.dma_start(red2[:], red_out[:])

            # B = Bp / denom (per class)
            rden = cp.tile([C7, 1], F32)
            nc.vector.reciprocal(rden[:], red2[:, 0:1])
            b_sb = cp.tile([C7, D], F32)
            nc.vector.tensor_scalar(out=b_sb[:], in0=red2[:, 1:1 + D], scalar1=rden[:],
                                    scalar2=None, op0=ALU.mult)
            nc.sync.dma_start(b_ext[:, :], b_sb[:])

            # A = P / denom: pre-copy P (overlaps the AllReduce), scale in place
            nc.vector.tensor_copy(a_full[:], p_all[:])
            den1 = cp.tile([1, C7], F32)
            nc.sync.dma_start(den1[:], red_out[:].rearrange("j o -> o j")[0:1, :])
            rden1 = cp.tile([1, C7], F32)
            nc.vector.reciprocal(rden1[:], den1[:])
            rb128 = cp.tile([128, C7], F32)
            nc.gpsimd.partition_broadcast(rb128[:], rden1[:])
            for t in range(rt):
                nc.vector.tensor_tensor(out=a_full[:, t, :], in0=a_full[:, t, :],
                                        in1=rb128[:], op=ALU.mult)
            nc.sync.dma_start(a_ext[:, :, :], a_full[:])

            # C = einsum(B, head_w) + head_b
            btf = cp.tile([128, DK, C7], F32)
            for k in range(DK):
                tp = ps_sm.tile([128, 128], F32, tag="sm")
                nc.tensor.transpose(tp[:, 0:C7], b_sb[:, k * 128:(k + 1) * 128],
                                    ident_f[0:C7, 0:C7])
                nc.vector.tensor_copy(btf[:, k, :], tp[:, 0:C7])
            c_ps = ps_sm.tile([1, C7], F32, tag="sm")
            first = True
            for kb in range(DK):
                for i in range(C7):
                    nc.tensor.matmul(c_ps[:], btf[:, kb, i:i + 1], hwt_sb[:, kb, i, :],
                                     start=first, stop=False)
                    first = False
            nc.tensor.matmul(c_ps[:], one_f[:], hb_sb[:], start=False, stop=True)
            c_sb = cp.tile([1, C7], F32)
            nc.vector.tensor_copy(c_sb[:], c_ps[:])
            nc.sync.dma_start(co_ext[:, :], c_sb[:])

    nc.compile()
    return nc


# --------------------------------------------------------------------------
# host-side prep / unshard
# --------------------------------------------------------------------------

def _prep_core_inputs(shard_x, shard_c, rt):
    """shard_x [rs, 1024] f32, shard_c [rs, 7] f32 -> device layouts."""
    import ml_dtypes
    rs = shard_x.shape[0]
    rpad = rt * 128
    xp = np.zeros((rpad, D), dtype=np.float32)
    xp[:rs] = shard_x
    # [t, p(i_in_tile), k(i_tile), r] ; value = x[t*128+r, k*128+p]
    xt = np.ascontiguousarray(
        xp.reshape(rt, 128, DK, 128).transpose(0, 3, 2, 1)
    ).astype(ml_dtypes.bfloat16)
    cp_ = np.full((rpad, C7), NEG_BIG, dtype=np.float32)
    cp_[:rs] = shard_c
    call = np.ascontiguousarray(cp_.reshape(rt, 128, C7).transpose(1, 0, 2))
    return xt, call


def _prep_shared_inputs(inp):
    import ml_dtypes
    bf = ml_dtypes.bfloat16
    out = {}
    for nm, key in (("wk", "key_w"), ("wv", "value_w"), ("wq", "query_w")):
        w = np.asarray(inp[key], dtype=np.float32)
        out[nm] = np.ascontiguousarray(w.T.reshape(DK, 128, D)).astype(bf)
    for nm, key in (("kb", "key_b"), ("vb", "value_b"), ("qb", "query_b")):
        out[nm] = np.asarray(inp[key], dtype=np.float32).reshape(1, D).astype(bf)
    hw = np.asarray(inp["head_w"], dtype=np.float32)  # [7, 7, 1024]
    out["hwt"] = np.ascontiguousarray(
        hw.transpose(2, 1, 0).reshape(DK, 128, C7, C7).transpose(1, 0, 2, 3)
    )
    out["hb"] = np.asarray(inp["head_b"], dtype=np.float32).reshape(1, C7)
    out["eye"] = np.tile(np.eye(C7, dtype=np.float32), (N_CORES, 1))
    return out


def kernel(**inputs) -> tuple:
    feats = np.asarray(inputs["features"], dtype=np.float32)
    c = np.asarray(inputs["c"], dtype=np.float32)
    n = feats.shape[0]
    assert n % N_CORES == 0
    rs = n // N_CORES
    rt = (rs + 127) // 128

    key = (rs, N_CORES)
    if key not in _BUILD_CACHE:
        _BUILD_CACHE[key] = build_graph(rs, N_CORES)
    nc = _BUILD_CACHE[key]

    shared = _prep_shared_inputs(inputs)
    in_maps = []
    for s in range(N_CORES):
        xt, call = _prep_core_inputs(feats[s * rs:(s + 1) * rs],
                                     c[s * rs:(s + 1) * rs], rt)
        m = {"xt": xt, "call": call}
        m.update(shared)
        in_maps.append(m)

    res = run_bass_kernel_spmd(nc, in_maps, core_ids=list(range(N_CORES)))

    a_parts = []
    for s in range(N_CORES):
        a_po = res.results[s]["A_out"]                      # [128, rt, 7]
        a_parts.append(a_po.transpose(1, 0, 2).reshape(rt * 128, C7)[:rs])
    A = np.concatenate(a_parts, axis=0).astype(np.float32)
    B = res.results[0]["B_out"].reshape(1, C7, D).astype(np.float32)
    Cc = res.results[0]["C_out"].reshape(1, C7).astype(np.float32)
    return (Cc, A, B)


# revision 33
# speedup vs baseline: 1.0191x; 1.0089x over previous
"""Trainium2 Bass kernel for nn_AttDual: dual-attention MIL pooling head.

Computation (see reference):
  K = chain(features, key_*)    ; chain = Linear -> LayerNorm -> GELU(erf)
  V = chain(K, value_*)
  Q = chain(K, query_*)
  top_idx = argmax(c, axis=0)   ; q_max = chain(K[top_idx], query_*)  (== Q[top_idx])
  A = softmax(Q @ q_max.T / 32, axis=0)   (column softmax over all N)
  B = A.T @ V ;  C = einsum('ik,oik->o', B, head_w) + head_b
Returns (C [1,7], A [N,7], B [1,7,1024]).

Distribution: data-parallel over N across 8 NeuronCores. Collectives:
one AllGather of [local-argmax value | selected K row] per class (each core
then picks the winners locally), and one AllReduce of [colsum | B].

Structure: pass A streams features and computes K tiles (K^T spilled to
DRAM) plus the local-argmax selection; pass B streams K^T back and fuses
the V/Q chains with logits -> exp -> B-accumulation per tile, lagged a few
tiles so the q_max chain hides under the head of pass B.

Matmuls in bf16 with f32 PSUM accumulation. LayerNorm stats via bn_stats;
rsqrt via DVE bit-trick + 2 Newton steps (keeps ScalarE on the Gelu table
through both passes; one switch to the exp table for the softmax).
"""
import numpy as np

import concourse.bacc as bacc
import concourse.tile as tile
import concourse.mybir as mybir
from concourse.bass_utils import run_bass_kernel_spmd

F32 = mybir.dt.float32
I32 = mybir.dt.int32
BF16 = mybir.dt.bfloat16
AF = mybir.ActivationFunctionType
ALU = mybir.AluOpType

N_CORES = 8
D = 1024
DK = 8          # d / 128
C7 = 7
EPS = 1e-5
NEG_BIG = -1.0e30

_BUILD_CACHE: dict = {}


def build_graph(rs: int, n_cores: int = N_CORES):
    """rs = rows per core (real). Tiles of 128 rows; last tile partial."""
    rt = (rs + 127) // 128
    vr_last = rs - (rt - 1) * 128
    ncp = n_cores * C7

    nc = bacc.Bacc("TRN2", target_bir_lowering=False, debug=False,
                   num_devices=n_cores)

    xt_ext = nc.declare_dram_parameter("xt", [rt, 128, DK, 128], BF16, isOutput=False)
    c_ext = nc.declare_dram_parameter("call", [128, rt, C7], F32, isOutput=False)
    wk_ext = nc.declare_dram_parameter("wk", [DK, 128, D], BF16, isOutput=False)
    wv_ext = nc.declare_dram_parameter("wv", [DK, 128, D], BF16, isOutput=False)
    wq_ext = nc.declare_dram_parameter("wq", [DK, 128, D], BF16, isOutput=False)
    kb_ext = nc.declare_dram_parameter("kb", [1, D], BF16, isOutput=False)
    vb_ext = nc.declare_dram_parameter("vb", [1, D], BF16, isOutput=False)
    qb_ext = nc.declare_dram_parameter("qb", [1, D], BF16, isOutput=False)
    hwt_ext = nc.declare_dram_parameter("hwt", [128, DK, C7, C7], F32, isOutput=False)
    hb_ext = nc.declare_dram_parameter("hb", [1, C7], F32, isOutput=False)
    eye_ext = nc.declare_dram_parameter("eye", [ncp, C7], F32, isOutput=False)

    a_ext = nc.declare_dram_parameter("A_out", [128, rt, C7], F32, isOutput=True)
    b_ext = nc.declare_dram_parameter("B_out", [C7, D], F32, isOutput=True)
    co_ext = nc.declare_dram_parameter("C_out", [1, C7], F32, isOutput=True)

    rg = [list(range(n_cores))]

    with tile.TileContext(nc) as tc:
        with (
            tc.tile_pool(name="const", bufs=1) as cp,
            tc.tile_pool(name="io", bufs=3) as iop,
            tc.tile_pool(name="act", bufs=2) as ap_,
            tc.tile_pool(name="qtp", bufs=15) as qtp,
            tc.tile_pool(name="vp", bufs=15) as vp,
            tc.tile_pool(name="sm", bufs=2) as smp,
            tc.tile_pool(name="ps_h", bufs=4, space="PSUM") as ps_h,
            tc.tile_pool(name="ps_acc", bufs=1, space="PSUM") as ps_acc,
            tc.tile_pool(name="ps_sm", bufs=2, space="PSUM") as ps_sm,
            tc.tile_pool(name="dram", bufs=1, space="DRAM") as dram,
        ):
            # ---------------- constants / pinned ----------------
            c_all = cp.tile([128, rt, C7], F32)
            nc.sync.dma_start(c_all[:], c_ext[:, :, :])
            wk_sb = [cp.tile([128, D], BF16, tag=f"wk{k}", name=f"wk_sb{k}") for k in range(DK)]
            wv_sb = [cp.tile([128, D], BF16, tag=f"wv{k}", name=f"wv_sb{k}") for k in range(DK)]
            wq_sb = [cp.tile([128, D], BF16, tag=f"wq{k}", name=f"wq_sb{k}") for k in range(DK)]
            for k in range(DK):
                nc.scalar.dma_start(wk_sb[k][:], wk_ext.ap()[k])
            for k in range(DK):
                nc.scalar.dma_start(wv_sb[k][:], wv_ext.ap()[k])
            for k in range(DK):
                nc.scalar.dma_start(wq_sb[k][:], wq_ext.ap()[k])
            bias_sb = cp.tile([65, D], BF16)
            nc.scalar.dma_start(bias_sb[0:1, :], kb_ext[:, :])
            nc.scalar.dma_start(bias_sb[32:33, :], vb_ext[:, :])
            nc.scalar.dma_start(bias_sb[64:65, :], qb_ext[:, :])
            kb_sb, vb_sb, qb_sb = bias_sb[0:1, :], bias_sb[32:33, :], bias_sb[64:65, :]
            hwt_sb = cp.tile([128, DK, C7, C7], F32)
            nc.scalar.dma_start(hwt_sb[:], hwt_ext[:, :, :, :])
            hb_sb = cp.tile([1, C7], F32)
            nc.scalar.dma_start(hb_sb[:], hb_ext[:, :])
            eye_f = cp.tile([ncp, C7], F32)
            nc.scalar.dma_start(eye_f[:], eye_ext[:, :])

            p_all = cp.tile([128, rt, C7], BF16)
            a_full = cp.tile([128, rt, C7], F32)

            ones_bf = cp.tile([65, 128], BF16)
            nc.gpsimd.memset(ones_bf[:], 1.0)
            one_f = cp.tile([1, 1], F32)
            nc.gpsimd.memset(one_f[:], 1.0)

            # identity matrices (via iota + is_equal)
            iota_p = cp.tile([128, 1], I32)
            nc.gpsimd.iota(iota_p[:], [[0, 1]], channel_multiplier=1)
            iota_pf = cp.tile([128, 1], F32)
            nc.vector.tensor_copy(iota_pf[:], iota_p[:])
            iota_f = cp.tile([128, 128], I32)
            nc.gpsimd.iota(iota_f[:], [[1, 128]], channel_multiplier=0)
            iota_ff = cp.tile([128, 128], F32)
            nc.vector.tensor_copy(iota_ff[:], iota_f[:])
            ident_f = cp.tile([128, 128], F32)
            nc.vector.tensor_scalar(out=ident_f[:], in0=iota_ff[:], scalar1=iota_pf[:],
                                    scalar2=None, op0=ALU.is_equal)
            ident_bf = cp.tile([128, 128], BF16)
            nc.vector.tensor_copy(ident_bf[:], ident_f[:])

            # collective + spill dram buffers
            ms_in = dram.tile([C7, 1 + D], F32)
            ms_out = dram.tile([ncp, 1 + D], F32, addr_space="Shared")
            gm_d = dram.tile([1, C7], F32)
            red_in = dram.tile([C7, 1 + D], F32)
            red_out = dram.tile([C7, 1 + D], F32, addr_space="Shared")
            k1t_dram = dram.tile([rt, 128, DK, 128], BF16)

            # ---------------- local column max of c ----------------
            cmax = cp.tile([128, C7], F32)
            nc.vector.tensor_reduce(out=cmax[:], in_=c_all[:].rearrange("p t j -> p j t"),
                                    axis=mybir.AxisListType.X, op=ALU.max)
            ctp = ps_sm.tile([C7, 128], F32, tag="sm")
            nc.tensor.transpose(ctp[:], cmax[:], ident_f[:])
            mloc = cp.tile([C7, 1], F32)
            nc.vector.tensor_reduce(out=mloc[:], in_=ctp[:], axis=mybir.AxisListType.X,
                                    op=ALU.max)
            mltp = ps_sm.tile([C7, 128], F32, tag="sm")
            nc.tensor.transpose(mltp[0:1, 0:C7], mloc[:], ident_f[0:C7, 0:C7])
            ml1 = cp.tile([1, C7], F32)
            nc.vector.tensor_copy(ml1[:], mltp[0:1, 0:C7])
            mg128 = cp.tile([128, C7], F32)
            nc.gpsimd.partition_broadcast(mg128[:], ml1[:])

            # ---------------- helpers ----------------
            def rsqrt_cols(vv, y, t1, t2, p, w):
                nc.vector.tensor_scalar(out=y[:p, :w].bitcast(I32),
                                        in0=vv[:p, :w].bitcast(I32),
                                        scalar1=1, scalar2=-1,
                                        op0=ALU.logical_shift_right,
                                        op1=ALU.bitwise_xor)
                nc.vector.tensor_scalar(out=y[:p, :w].bitcast(I32),
                                        in0=y[:p, :w].bitcast(I32),
                                        scalar1=0x5F3759E0, scalar2=None, op0=ALU.add)
                for _ in range(2):
                    nc.vector.tensor_tensor(out=t1[:p, :w], in0=y[:p, :w],
                                            in1=y[:p, :w], op=ALU.mult)
                    nc.vector.tensor_tensor(out=t2[:p, :w], in0=t1[:p, :w],
                                            in1=vv[:p, :w], op=ALU.mult)
                    nc.vector.tensor_scalar(out=t2[:p, :w], in0=t2[:p, :w], scalar1=-0.5,
                                            scalar2=1.5, op0=ALU.mult, op1=ALU.add)
                    nc.vector.tensor_tensor(out=y[:p, :w], in0=y[:p, :w],
                                            in1=t2[:p, :w], op=ALU.mult)

            def ln_gelu2(hA, hB, outA, outB, p, tagp):
                """gelu((h - mean)/sqrt(var+eps)) over two [p, 512] psum halves."""
                bns = smp.tile([128, 12], F32, tag=f"bns{tagp}")
                mv = smp.tile([128, 2], F32, tag=f"mv{tagp}")
                vv = smp.tile([128, 1], F32, tag=f"vv{tagp}")
                y = smp.tile([128, 1], F32, tag=f"y{tagp}")
                t1 = smp.tile([128, 1], F32, tag=f"t1{tagp}")
                t2 = smp.tile([128, 1], F32, tag=f"t2{tagp}")
                nb = smp.tile([128, 1], F32, tag=f"nb{tagp}")
                nc.vector.bn_stats(bns[:p, 0:6], hA)
                nc.vector.bn_stats(bns[:p, 6:12], hB)
                nc.vector.bn_aggr(mv[:p, :], bns[:p, :])
                nc.vector.tensor_scalar(out=vv[:p, :], in0=mv[:p, 1:2], scalar1=EPS,
                                        scalar2=None, op0=ALU.add)
                rsqrt_cols(vv, y, t1, t2, p, 1)
                nc.vector.scalar_tensor_tensor(out=nb[:p, :], in0=mv[:p, 0:1],
                                               scalar=-1.0, in1=y[:p, :],
                                               op0=ALU.mult, op1=ALU.mult)
                nc.scalar.activation(outA, hA, AF.Gelu, bias=nb[:p, :], scale=y[:p, :])
                nc.scalar.activation(outB, hB, AF.Gelu, bias=nb[:p, :], scale=y[:p, :])

            def chain_mms(tgt, lhs_fn, w_sb, b_sb, p):
                for oc in range(2):
                    sl = slice(oc * 512, (oc + 1) * 512)
                    for k in range(DK):
                        nc.tensor.matmul(tgt(oc), lhs_fn(k), w_sb[k][:, sl],
                                         start=(k == 0), stop=False)
                    bp_ = b_sb.base_partition()
                    nc.tensor.matmul(tgt(oc), ones_bf[bp_:bp_ + 1, :p],
                                     b_sb[:1, sl], start=False, stop=True)

            # ================= pass A: K chains + local selection =============
            msel_ps = ps_acc.tile([C7, D], F32, tag="acc")
            for t in range(rt):
                xt_t = iop.tile([128, DK, 128], BF16, tag="xt")
                nc.gpsimd.dma_start(xt_t[:], xt_ext.ap()[t])
                hh = [ps_h.tile([128, 512], F32, tag="h", name=f"hk{t}_{o}")
                      for o in range(2)]
                chain_mms(lambda oc: hh[oc][:, :], lambda k: xt_t[:, k, :],
                          wk_sb, kb_sb, 128)
                k1 = ap_.tile([128, D], BF16, tag="k1")
                ln_gelu2(hh[0][:, :], hh[1][:, :], k1[:, 0:512], k1[:, 512:1024],
                         128, "m")
                pvr = vr_last if t == rt - 1 else 128
                oh = smp.tile([128, C7], BF16, tag="oh")
                nc.vector.tensor_tensor(out=oh[:], in0=c_all[:, t, :], in1=mg128[:],
                                        op=ALU.is_equal)
                for oc in range(2):
                    sl = slice(oc * 512, (oc + 1) * 512)
                    nc.tensor.matmul(msel_ps[:, sl], oh[:pvr, :], k1[:pvr, sl],
                                     start=(t == 0), stop=(t == rt - 1))
                k1t = ap_.tile([128, DK, 128], BF16, tag="k1t")
                nc.sync.dma_start(k1t[:], k1[:], transpose=True)
                nc.gpsimd.dma_start(k1t_dram[t], k1t[:])

            # ======== m_sel gather + winner selection + q_max chain ==========
            msc = cp.tile([C7, 1 + D], F32)
            nc.vector.tensor_copy(msc[:, 0:1], mloc[:])
            nc.vector.tensor_copy(msc[:, 1:1 + D], msel_ps[:])
            nc.sync.dma_start(ms_in[:], msc[:])
            nc.gpsimd.collective_compute(
                "AllGather", ALU.bypass, replica_groups=rg,
                ins=[ms_in[:].opt()], outs=[ms_out[:].opt()])
            g_sb = cp.tile([ncp, 1 + D], F32)
            nc.sync.dma_start(g_sb[:], ms_out[:])
            mv78 = cp.tile([C7, n_cores], F32)
            nc.sync.dma_start(mv78[:],
                              ms_out[:].rearrange("(s j) o -> j s o", j=C7)[:, :, 0])
            gmax = cp.tile([C7, 1], F32)
            nc.vector.tensor_reduce(out=gmax[:], in_=mv78[:],
                                    axis=mybir.AxisListType.X, op=ALU.max)
            nc.sync.dma_start(gm_d[:], gmax[:])
            gmrep = cp.tile([ncp, 1], F32)
            nc.sync.dma_start(gmrep[:], gm_d[0:1, :].partition_broadcast(n_cores))
            mv56 = cp.tile([ncp, 1], F32)
            nc.sync.dma_start(mv56[:], ms_out[:, 0:1])
            masks = cp.tile([ncp, 1], F32)
            nc.vector.tensor_scalar(out=masks[:], in0=mv56[:], scalar1=gmrep[:],
                                    scalar2=None, op0=ALU.is_equal)
            sel_bf = cp.tile([ncp, C7], BF16)
            nc.vector.tensor_scalar(out=sel_bf[:], in0=eye_f[:], scalar1=masks[:],
                                    scalar2=None, op0=ALU.mult)
            g_bf = cp.tile([ncp, D], BF16)
            nc.vector.tensor_copy(g_bf[:], g_sb[:, 1:1 + D])
            mselT_ps = ps_sm.tile([128, C7 * DK], F32, tag="sm")
            for ob in range(DK):
                nc.tensor.matmul(mselT_ps[:, ob * C7:(ob + 1) * C7],
                                 g_bf[:, ob * 128:(ob + 1) * 128], sel_bf[:],
                                 start=True, stop=True)
            mselT = cp.tile([128, DK, C7], BF16)
            nc.vector.tensor_copy(mselT[:], mselT_ps[:])

            hq_ps = ps_acc.tile([C7, D], F32, tag="acc")
            chain_mms(lambda oc: hq_ps[:C7, oc * 512:(oc + 1) * 512],
                      lambda k: mselT[:, k, :], wq_sb, qb_sb, C7)
            qmax = cp.tile([C7, D], BF16)
            ln_gelu2(hq_ps[:C7, 0:512], hq_ps[:C7, 512:1024],
                     qmax[:, 0:512], qmax[:, 512:1024], C7, "q")
            qmaxT = cp.tile([128, DK, C7], BF16)
            for k in range(DK):
                tpb = ps_sm.tile([128, 128], BF16, tag="sm")
                nc.tensor.transpose(tpb[:, 0:C7], qmax[:, k * 128:(k + 1) * 128],
                                    ident_bf[0:C7, 0:C7])
                nc.vector.tensor_copy(qmaxT[:, k, :], tpb[:, 0:C7])

            # ======== pass B: V/Q chains fused with logits/exp/Bp ============
            bp_ps = ps_acc.tile([C7, D], F32, tag="acc")
            cs_acc = cp.tile([C7, 2], F32)
            nc.gpsimd.memset(cs_acc[:], 0.0)
            qt_tiles = {}
            v_tiles = {}
            LAGB = 5

            def emit_vq(t):
                k1t_t = iop.tile([128, DK, 128], BF16, tag="k1tl")
                nc.gpsimd.dma_start(k1t_t[:], k1t_dram[t])
                hv = [ps_h.tile([128, 512], F32, tag="h", name=f"hv{t}_{o}")
                      for o in range(2)]
                chain_mms(lambda oc: hv[oc][:, :], lambda k: k1t_t[:, k, :],
                          wv_sb, vb_sb, 128)
                v_t = vp.tile([128, D], BF16, tag="v")
                ln_gelu2(hv[0][:, :], hv[1][:, :], v_t[:, 0:512], v_t[:, 512:1024],
                         128, "m")
                hq = [ps_h.tile([128, 512], F32, tag="h", name=f"hqv{t}_{o}")
                      for o in range(2)]
                chain_mms(lambda oc: hq[oc][:, :], lambda k: k1t_t[:, k, :],
                          wq_sb, qb_sb, 128)
                q1 = ap_.tile([128, D], BF16, tag="q1")
                ln_gelu2(hq[0][:, :], hq[1][:, :], q1[:, 0:512], q1[:, 512:1024],
                         128, "m")
                qt = qtp.tile([128, DK, 128], BF16, tag="qt")
                nc.scalar.dma_start(qt[:], q1[:], transpose=True)
                qt_tiles[t] = qt
                v_tiles[t] = v_t

            def emit_att(t):
                qt = qt_tiles.pop(t)
                v_t = v_tiles.pop(t)
                pvr = vr_last if t == rt - 1 else 128
                lg = ps_sm.tile([C7, 128], F32, tag="sm", name=f"lg{t}")
                for k in range(DK):
                    nc.tensor.matmul(lg[:C7, :], qmaxT[:, k, :], qt[:, k, :],
                                     start=(k == 0), stop=(k == DK - 1))
                pt_t = smp.tile([C7, 128], BF16, tag="ptt")
                nc.scalar.activation(pt_t[:], lg[:], AF.Exp,
                                     scale=float(1.0 / np.sqrt(D)))
                nc.vector.tensor_reduce(out=cs_acc[:, 1:2], in_=pt_t[:, 0:pvr],
                                        axis=mybir.AxisListType.X, op=ALU.add)
                nc.vector.tensor_tensor(out=cs_acc[:, 0:1], in0=cs_acc[:, 0:1],
                                        in1=cs_acc[:, 1:2], op=ALU.add)
                ptp = ps_sm.tile([128, 128], BF16, tag="sm", name=f"ptp{t}")
                nc.tensor.transpose(ptp[:, 0:C7], pt_t[:C7, :], ident_bf[0:C7, 0:C7])
                nc.vector.tensor_copy(p_all[:, t, :], ptp[:, 0:C7])
                for oc in range(2):
                    sl = slice(oc * 512, (oc + 1) * 512)
                    nc.tensor.matmul(bp_ps[:, sl], p_all[:pvr, t, :], v_t[:pvr, sl],
                                     start=(t == 0), stop=(t == rt - 1))

            # emit attention stages in groups of G so the exp activations run
            # back-to-back (2 ScalarE table loads per group, not per tile)
            G = 8
            ready = []
            done_att = 0
            for t in range(rt):
                emit_vq(t)
                if t >= LAGB:
                    ready.append(t - LAGB)
                if len(ready) >= G:
                    for u in ready:
                        emit_att(u)
                    done_att += len(ready)
                    ready = []
            for u in ready:
                emit_att(u)
            for t in range(rt - LAGB, rt):
                emit_att(t)

            # ---------------- final reduce: [colsum | Bp] ----------------
            red_sb = cp.tile([C7, 1 + D], F32)
            nc.vector.tensor_copy(red_sb[:, 0:1], cs_acc[:, 0:1])
            nc.vector.tensor_copy(red_sb[:, 1:1 + D], bp_ps[:])
            nc.sync.dma_start(red_in[:], red_sb[:])
            nc.gpsimd.collective_compute(
                "AllReduce", ALU.add, replica_groups=rg,
                ins=[red_in[:].opt()], outs=[red_out[:].opt()])
            red2 = red_sb
            nc.sync.dma_start(red2[:], red_out[:])

            # B = Bp / denom (per class)
            rden = cp.tile([C7, 1], F32)
            nc.vector.reciprocal(rden[:], red2[:, 0:1])
            b_sb = cp.tile([C7, D], F32)
            nc.vector.tensor_scalar(out=b_sb[:], in0=red2[:, 1:1 + D], scalar1=rden[:],
                                    scalar2=None, op0=ALU.mult)
            nc.sync.dma_start(b_ext[:, :], b_sb[:])

            # A = P / denom: pre-copy P (overlaps the AllReduce), scale in place
            nc.vector.tensor_copy(a_full[:], p_all[:])
            den1 = cp.tile([1, C7], F32)
            nc.sync.dma_start(den1[:], red_out[:].rearrange("j o -> o j")[0:1, :])
            rden1 = cp.tile([1, C7], F32)
            nc.vector.reciprocal(rden1[:], den1[:])
            rb128 = cp.tile([128, C7], F32)
            nc.gpsimd.partition_broadcast(rb128[:], rden1[:])
            for t in range(rt):
                nc.vector.tensor_tensor(out=a_full[:, t, :], in0=a_full[:, t, :],
                                        in1=rb128[:], op=ALU.mult)
            nc.sync.dma_start(a_ext[:, :, :], a_full[:])

            # C = einsum(B, head_w) + head_b
            btf = cp.tile([128, DK, C7], F32)
            for k in range(DK):
                tp = ps_sm.tile([128, 128], F32, tag="sm")
                nc.tensor.transpose(tp[:, 0:C7], b_sb[:, k * 128:(k + 1) * 128],
                                    ident_f[0:C7, 0:C7])
                nc.vector.tensor_copy(btf[:, k, :], tp[:, 0:C7])
            c_ps = ps_sm.tile([1, C7], F32, tag="sm")
            first = True
            for kb in range(DK):
                for i in range(C7):
                    nc.tensor.matmul(c_ps[:], btf[:, kb, i:i + 1], hwt_sb[:, kb, i, :],
                                     start=first, stop=False)
                    first = False
            nc.tensor.matmul(c_ps[:], one_f[:], hb_sb[:], start=False, stop=True)
            c_sb = cp.tile([1, C7], F32)
            nc.vector.tensor_copy(c_sb[:], c_ps[:])
            nc.sync.dma_start(co_ext[:, :], c_sb[:])

    nc.compile()
    return nc


# --------------------------------------------------------------------------
# host-side prep / unshard
# --------------------------------------------------------------------------

def _prep_core_inputs(shard_x, shard_c, rt):
    """shard_x [rs, 1024] f32, shard_c [rs, 7] f32 -> device layouts."""
    import ml_dtypes
    rs = shard_x.shape[0]
    rpad = rt * 128
    xp = np.zeros((rpad, D), dtype=np.float32)
    xp[:rs] = shard_x
    # [t, p(i_in_tile), k(i_tile), r] ; value = x[t*128+r, k*128+p]
    xt = np.ascontiguousarray(
        xp.reshape(rt, 128, DK, 128).transpose(0, 3, 2, 1)
    ).astype(ml_dtypes.bfloat16)
    cp_ = np.full((rpad, C7), NEG_BIG, dtype=np.float32)
    cp_[:rs] = shard_c
    call = np.ascontiguousarray(cp_.reshape(rt, 128, C7).transpose(1, 0, 2))
    return xt, call


def _prep_shared_inputs(inp):
    import ml_dtypes
    bf = ml_dtypes.bfloat16
    out = {}
    for nm, key in (("wk", "key_w"), ("wv", "value_w"), ("wq", "query_w")):
        w = np.asarray(inp[key], dtype=np.float32)
        out[nm] = np.ascontiguousarray(w.T.reshape(DK, 128, D)).astype(bf)
    for nm, key in (("kb", "key_b"), ("vb", "value_b"), ("qb", "query_b")):
        out[nm] = np.asarray(inp[key], dtype=np.float32).reshape(1, D).astype(bf)
    hw = np.asarray(inp["head_w"], dtype=np.float32)  # [7, 7, 1024]
    out["hwt"] = np.ascontiguousarray(
        hw.transpose(2, 1, 0).reshape(DK, 128, C7, C7).transpose(1, 0, 2, 3)
    )
    out["hb"] = np.asarray(inp["head_b"], dtype=np.float32).reshape(1, C7)
    out["eye"] = np.tile(np.eye(C7, dtype=np.float32), (N_CORES, 1))
    return out


def kernel(**inputs) -> tuple:
    feats = np.asarray(inputs["features"], dtype=np.float32)
    c = np.asarray(inputs["c"], dtype=np.float32)
    n = feats.shape[0]
    assert n % N_CORES == 0
    rs = n // N_CORES
    rt = (rs + 127) // 128

    key = (rs, N_CORES)
    if key not in _BUILD_CACHE:
        _BUILD_CACHE[key] = build_graph(rs, N_CORES)
    nc = _BUILD_CACHE[key]

    shared = _prep_shared_inputs(inputs)
    in_maps = []
    for s in range(N_CORES):
        xt, call = _prep_core_inputs(feats[s * rs:(s + 1) * rs],
                                     c[s * rs:(s + 1) * rs], rt)
        m = {"xt": xt, "call": call}
        m.update(shared)
        in_maps.append(m)

    res = run_bass_kernel_spmd(nc, in_maps, core_ids=list(range(N_CORES)))

    a_parts = []
    for s in range(N_CORES):
        a_po = res.results[s]["A_out"]                      # [128, rt, 7]
        a_parts.append(a_po.transpose(1, 0, 2).reshape(rt * 128, C7)[:rs])
    A = np.concatenate(a_parts, axis=0).astype(np.float32)
    B = res.results[0]["B_out"].reshape(1, C7, D).astype(np.float32)
    Cc = res.results[0]["C_out"].reshape(1, C7).astype(np.float32)
    return (Cc, A, B)
